# revision 3
# baseline (speedup 1.0000x reference)
"""Trainium2 Bass kernel for nn_Conv_agg (edge-parallel GNN message passing).

Math (see reference):
    out[n] = sum_k ( sum_{e: src(e)=n} X[e,k] * h[tgt(e)] ) @ W[k] + bias

Structure exploited (asserted at runtime, guaranteed by setup_inputs):
  - src(e) = e // DEG exactly (each node emits DEG=16 consecutive edges)
  - edges/nodes of graph g are contiguous and tgt(e) stays inside graph g's
    100-node window -> the whole problem is block-diagonal over graphs.

Dense per-graph formulation (no gather at all):
    M_k[s,t] = sum_{e in seg(s), tgt_e=t} X[e,k]      (100x100 per graph, per k)
    out_g    = sum_k M_k @ (h_g @ W_k) + bias

Per-core device pipeline (125 graphs/core, all bf16 on the PE):
  1. DVE:  O[e,t] = (tgt_e == t) one-hot via is_equal vs iota const
  2. Pool: Xall[e,(s,k)] = X[e,k] * blockdiag_mask (8 sources per 128-edge blk)
  3. PE:   M^T[t,(s,k)] block = O_b^T @ Xall_b, 13 blocks of 128 edges
  4. Act:  copy M^T PSUM -> SBUF bf16, de-interleaving k
  5. PE:   hW[t,(k,o)] = h_g^T.T @ [W0|W1]   (h^T preloaded, host-transposed)
  6. Pool: copy hW PSUM -> SBUF bf16
  7. PE:   out[s,o] = sum_k M_k^T.T @ hW_k   (PSUM accumulate over k)
  8. DVE adds bias (f32), DMA out rows.
"""

import numpy as np

B, NPG, DEG, K, CIN, COUT = 1000, 100, 16, 2, 128, 128
E = B * NPG * DEG            # 1,600,000 edges
NT = B * NPG                 # 100,000 nodes
NCORES = 8
G_C = B // NCORES            # 125 graphs / core
NT_C = NT // NCORES          # 12,500 nodes / core
E_C = E // NCORES            # 200,000 edges / core
EPG = NPG * DEG              # 1600 edges / graph
NB = -(-EPG // 128)          # 13 blocks of 128 edges (last half-padded)
EPG_P = NB * 128             # 1664
SPB = 128 // DEG             # 8 sources per 128-edge block
S_P = NB * SPB               # 104 source slots (100 real + 4 pad)

_module_cache = {}


def _patch_tile_drain():
    """This walrus build allows a single sync-wait per instruction; Tile's
    kernel-tail drain aggregates one wait per outstanding sem onto one
    InstDrain. Hoist extras onto dedicated sync nops (sequential on SP)."""
    import concourse.mybir as mybir
    from concourse.tile import TileContext
    from concourse.vector_clock import ScopedClock

    if getattr(TileContext, "_drain_patched", False):
        return

    def _drain_and_barrier(self, tick_clock, wait_clock):
        probe = self.nc.sync.nop(nofuse=True)
        wait_clock.add_sem_waits(probe.ins, ScopedClock({None: tick_clock.global_clock}))
        si = probe.ins.sync_info
        waits = list(si.on_wait) if si is not None and si.on_wait else []
        if si is not None and len(waits) > 1:
            si.on_wait = waits[:1]
            for w in waits[1:]:
                n = self.nc.sync.nop(nofuse=True)
                n.ins.sync_info = mybir.SyncInfo(on_wait=[w], on_update=[])
        self.nc.sync.drain()
        self.nc.all_engine_barrier()
        assert self.sems is not None
        popped = self.nc._tile_sem_poison_stack.pop()
        assert popped is self._sem_poison
        self.nc.clear_and_free_semaphores(list(self.sems.allocated().values()))
        self.nc.all_engine_barrier()

    TileContext._drain_and_barrier = _drain_and_barrier
    TileContext._drain_patched = True


def _build_module():
    import concourse.bacc as bacc
    import concourse.mybir as mybir
    from concourse.tile import TileContext

    _patch_tile_drain()
    f32 = mybir.dt.float32
    bf16 = mybir.dt.bfloat16

    nc = bacc.Bacc("TRN2", target_bir_lowering=False)
    ht_t = nc.dram_tensor("ht", [CIN, NT_C], bf16, kind="ExternalInput")
    xr_t = nc.dram_tensor("xr", [128, G_C, NB, K], bf16, kind="ExternalInput")
    tg_t = nc.dram_tensor("tg", [128, G_C, NB], bf16, kind="ExternalInput")
    w_t = nc.dram_tensor("w", [CIN, K, COUT], bf16, kind="ExternalInput")
    mask_t = nc.dram_tensor("mask", [128, SPB], bf16, kind="ExternalInput")
    iota_t = nc.dram_tensor("iota", [128, NPG], bf16, kind="ExternalInput")
    bias_t = nc.dram_tensor("bias", [128, COUT], f32, kind="ExternalInput")
    out_t = nc.dram_tensor("out", [NT_C, COUT], f32, kind="ExternalOutput")

    with TileContext(nc) as tc:
        with (
            tc.tile_pool(name="consts", bufs=1) as cpool,
            tc.tile_pool(name="op", bufs=3) as opool,
            tc.tile_pool(name="xap", bufs=3) as xapool,
            tc.tile_pool(name="mtp", bufs=3) as mtpool,
            tc.tile_pool(name="hwp", bufs=3) as hwpool,
            tc.tile_pool(name="outp", bufs=3) as outpool,
            tc.tile_pool(name="psM", bufs=2, space="PSUM") as psM,
            tc.tile_pool(name="psH", bufs=2, space="PSUM") as psH,
            tc.tile_pool(name="psO", bufs=2, space="PSUM") as psO,
        ):
            ht_sb = cpool.tile([CIN, NT_C], bf16)
            nc.sync.dma_start(ht_sb[:, :], ht_t[:, :])
            xr_sb = cpool.tile([128, G_C, NB, K], bf16)
            nc.sync.dma_start(xr_sb[:, :, :, :], xr_t[:, :, :, :])
            tg_sb = cpool.tile([128, G_C, NB], bf16)
            nc.sync.dma_start(tg_sb[:, :, :], tg_t[:, :, :])
            w_sb = cpool.tile([CIN, K, COUT], bf16)
            nc.sync.dma_start(w_sb[:, :, :], w_t[:, :, :])
            mask_sb = cpool.tile([128, SPB], bf16)
            nc.sync.dma_start(mask_sb[:, :], mask_t[:, :])
            iota_sb = cpool.tile([128, NPG], bf16)
            nc.sync.dma_start(iota_sb[:, :], iota_t[:, :])
            bias_sb = cpool.tile([128, COUT], f32)
            nc.sync.dma_start(bias_sb[:, :], bias_t[:, :])

            for g in range(G_C):
                # 1. one-hot O[e_part, blk, t] = (tgt == t)
                o_sb = opool.tile([128, NB, NPG], bf16)
                nc.vector.tensor_tensor(
                    o_sb[:, :, :],
                    tg_sb[:, g, :].unsqueeze(2).broadcast_to([128, NB, NPG]),
                    iota_sb[:, :].unsqueeze(1).broadcast_to([128, NB, NPG]),
                    op=mybir.AluOpType.is_equal,
                )
                # 2. Xall[e_part, blk, s, k] = X * (e_part//16 == s)
                xa_sb = xapool.tile([128, NB, SPB, K], bf16)
                nc.gpsimd.tensor_tensor(
                    xa_sb[:, :, :, :],
                    xr_sb[:, g, :, :].unsqueeze(2).broadcast_to([128, NB, SPB, K]),
                    mask_sb[:, :].unsqueeze(1).unsqueeze(3)
                        .broadcast_to([128, NB, SPB, K]),
                    op=mybir.AluOpType.mult,
                )
                # 3. M^T[t, blk, (s,k)] = O_b^T @ Xall_b per 128-edge block
                psM_tl = psM.tile([NPG, NB, SPB * K], f32)
                for b in range(NB):
                    nc.tensor.matmul(
                        psM_tl[:, b, :],
                        o_sb[:, b, :],           # lhsT [128e, 100t]
                        xa_sb[:, b, :, :],       # rhs  [128e, 16]
                        start=True, stop=True,
                    )
                # 4. PSUM -> SBUF bf16, de-interleave k: [t, k, (b s)]
                mt_sb = mtpool.tile([NPG, K, NB, SPB], bf16)
                nc.scalar.copy(
                    mt_sb[:, :, :, :].rearrange("p k b s -> p b s k"),
                    psM_tl[:, :, :].rearrange("p b (s k) -> p b s k", k=K),
                )
                # 5. hW[t, (k,o)] = h_g @ [W0|W1]
                psH_tl = psH.tile([NPG, K, COUT], f32)
                nc.tensor.matmul(
                    psH_tl[:, :, :].rearrange("p k o -> p (k o)"),
                    ht_sb[:, g * NPG:(g + 1) * NPG],   # lhsT [128c, 100t]
                    w_sb[:, :, :].rearrange("c k o -> c (k o)"),
                    start=True, stop=True,
                )
                hw_sb = hwpool.tile([NPG, K, COUT], bf16)
                nc.scalar.copy(hw_sb[:, :, :], psH_tl[:, :, :])
                # 7. out[s, o] += M_k^T.T @ hW_k
                psO_tl = psO.tile([S_P, COUT], f32)
                for k in range(K):
                    nc.tensor.matmul(
                        psO_tl[:, :],
                        mt_sb[:, k, :, :].rearrange("p b s -> p (b s)"),
                        hw_sb[:, k, :],
                        start=(k == 0), stop=(k == K - 1),
                    )
                # 8. bias + store
                o_out = outpool.tile([NPG, COUT], f32)
                nc.vector.tensor_tensor(o_out[:, :], psO_tl[:NPG, :],
                                        bias_sb[:NPG, :], op=mybir.AluOpType.add)
                nc.sync.dma_start(out_t[g * NPG:(g + 1) * NPG, :], o_out[:, :])
    nc.compile()
    return nc


def _get_module():
    if "nc" not in _module_cache:
        _module_cache["nc"] = _build_module()
    return _module_cache["nc"]


def _prep_inputs(h, X, tgt, weight, bias):
    """Host-side sharding/layout (no arithmetic on data values)."""
    import ml_dtypes
    bf16 = ml_dtypes.bfloat16

    g_all = np.arange(E, dtype=np.int64) // EPG      # graph id per edge
    tloc = tgt - g_all * NPG                         # within-graph target
    assert tloc.min() >= 0 and tloc.max() < NPG, "tgt escapes graph block"

    tlp = np.zeros((NCORES, G_C, EPG_P), np.float32)
    tlp[:, :, :EPG] = tloc.reshape(NCORES, G_C, EPG)
    Xp = np.zeros((NCORES, G_C, EPG_P, K), np.float32)
    Xp[:, :, :EPG] = X.reshape(NCORES, G_C, EPG, K)

    # e = 128*b + p  ->  [core, p, g, b(, k)]
    tg_arr = np.ascontiguousarray(
        tlp.reshape(NCORES, G_C, NB, 128).transpose(0, 3, 1, 2)).astype(bf16)
    xr_arr = np.ascontiguousarray(
        Xp.reshape(NCORES, G_C, NB, 128, K).transpose(0, 3, 1, 2, 4)).astype(bf16)

    ht = np.ascontiguousarray(
        h.astype(bf16).reshape(NCORES, NT_C, CIN).transpose(0, 2, 1))

    iota = np.ascontiguousarray(
        np.broadcast_to(np.arange(NPG, dtype=np.float32), (128, NPG))).astype(bf16)
    mask = (np.arange(128)[:, None] // DEG
            == np.arange(SPB)[None, :]).astype(bf16)
    w2 = np.ascontiguousarray(weight.transpose(1, 0, 2)).astype(bf16)
    bias_rep = np.ascontiguousarray(
        np.broadcast_to(bias, (128, COUT))).astype(np.float32)
    return ht, xr_arr, tg_arr, w2, mask, iota, bias_rep


def kernel(h, X, edge_index, node_index, batch_node, batch_edge, num_node,
           weight, bias):
    from concourse.bass_utils import run_bass_kernel_spmd

    h = np.asarray(h, np.float32)
    X = np.asarray(X, np.float32)
    edge_index = np.asarray(edge_index)
    weight = np.asarray(weight, np.float32)
    bias = np.asarray(bias, np.float32)

    src = np.asarray(edge_index[1])
    tgt = np.asarray(edge_index[2])
    # structural contract from setup_inputs (see module docstring)
    assert src.shape == (E,) and h.shape == (NT, CIN) and X.shape == (E, K)
    assert np.array_equal(src, np.arange(E, dtype=src.dtype) // DEG), \
        "edges not sorted as src=e//DEG"

    ht, xr_arr, tg_arr, w2, mask, iota, bias_rep = _prep_inputs(
        h, X, tgt, weight, bias)

    nc = _get_module()
    in_maps = []
    for c in range(NCORES):
        in_maps.append({
            "ht": ht[c],
            "xr": xr_arr[c],
            "tg": tg_arr[c],
            "w": w2,
            "mask": mask,
            "iota": iota,
            "bias": bias_rep,
        })
    res = run_bass_kernel_spmd(nc, in_maps, core_ids=list(range(NCORES)))
    out = np.concatenate([r["out"] for r in res.results], axis=0)
    return out


# revision 8
# speedup vs baseline: 1.2117x; 1.2117x over previous
"""Trainium2 Bass kernel for nn_Conv_agg (edge-parallel GNN message passing).

Math (see reference):
    out[n] = sum_k ( sum_{e: src(e)=n} X[e,k] * h[tgt(e)] ) @ W[k] + bias

Structure exploited (asserted at runtime, guaranteed by setup_inputs):
  - src(e) = e // DEG exactly (each node emits DEG=16 consecutive edges)
  - edges/nodes of graph g are contiguous and tgt(e) stays inside graph g's
    100-node window -> the whole problem is block-diagonal over graphs.

Dense per-graph formulation (no gather at all):
    M_k[s,t] = sum_{e in seg(s), tgt_e=t} X[e,k]      (100x100 per graph, per k)
    out_g    = sum_k M_k @ (h_g @ W_k) + bias

Per-core device pipeline (125 graphs/core, all bf16 on the PE):
  1. DVE:  O[e,t] = (tgt_e == t) one-hot via is_equal vs iota const
  2. Pool: Xall[e,(s,k)] = X[e,k] * blockdiag_mask (8 sources per 128-edge blk)
  3. PE:   M^T[t,(s,k)] block = O_b^T @ Xall_b, 13 blocks of 128 edges
  4. Act:  copy M^T PSUM -> SBUF bf16, de-interleaving k
  5. PE:   hW[t,(k,o)] = h_g^T.T @ [W0|W1]   (h^T preloaded, host-transposed)
  6. Pool: copy hW PSUM -> SBUF bf16
  7. PE:   out[s,o] = sum_k M_k^T.T @ hW_k   (PSUM accumulate over k)
  8. DVE adds bias (f32), DMA out rows.
"""

import numpy as np

B, NPG, DEG, K, CIN, COUT = 1000, 100, 16, 2, 128, 128
E = B * NPG * DEG            # 1,600,000 edges
NT = B * NPG                 # 100,000 nodes
NCORES = 8
G_C = B // NCORES            # 125 graphs / core
NT_C = NT // NCORES          # 12,500 nodes / core
E_C = E // NCORES            # 200,000 edges / core
EPG = NPG * DEG              # 1600 edges / graph
NB = -(-EPG // 128)          # 13 blocks of 128 edges (last half-padded)
EPG_P = NB * 128             # 1664
SPB = 128 // DEG             # 8 sources per 128-edge block
S_P = NB * SPB               # 104 source slots (100 real + 4 pad)

_module_cache = {}


def _patch_tile_drain():
    """This walrus build allows a single sync-wait per instruction; Tile's
    kernel-tail drain aggregates one wait per outstanding sem onto one
    InstDrain. Hoist extras onto dedicated sync nops (sequential on SP)."""
    import concourse.mybir as mybir
    from concourse.tile import TileContext
    from concourse.vector_clock import ScopedClock

    if getattr(TileContext, "_drain_patched", False):
        return

    def _drain_and_barrier(self, tick_clock, wait_clock):
        probe = self.nc.sync.nop(nofuse=True)
        wait_clock.add_sem_waits(probe.ins, ScopedClock({None: tick_clock.global_clock}))
        si = probe.ins.sync_info
        waits = list(si.on_wait) if si is not None and si.on_wait else []
        if si is not None and len(waits) > 1:
            si.on_wait = waits[:1]
            for w in waits[1:]:
                n = self.nc.sync.nop(nofuse=True)
                n.ins.sync_info = mybir.SyncInfo(on_wait=[w], on_update=[])
        self.nc.sync.drain()
        self.nc.all_engine_barrier()
        assert self.sems is not None
        popped = self.nc._tile_sem_poison_stack.pop()
        assert popped is self._sem_poison
        self.nc.clear_and_free_semaphores(list(self.sems.allocated().values()))
        self.nc.all_engine_barrier()

    TileContext._drain_and_barrier = _drain_and_barrier
    TileContext._drain_patched = True


def _build_module():
    import concourse.bacc as bacc
    import concourse.mybir as mybir
    from concourse.tile import TileContext

    _patch_tile_drain()
    f32 = mybir.dt.float32
    bf16 = mybir.dt.bfloat16

    nc = bacc.Bacc("TRN2", target_bir_lowering=False)
    ht_t = nc.dram_tensor("ht", [CIN, NT_C], bf16, kind="ExternalInput")
    xr_t = nc.dram_tensor("xr", [128, G_C, NB, K], bf16, kind="ExternalInput")
    tg_t = nc.dram_tensor("tg", [128, G_C, NB], bf16, kind="ExternalInput")
    w_t = nc.dram_tensor("w", [CIN, K, COUT], bf16, kind="ExternalInput")
    mask_t = nc.dram_tensor("mask", [128, SPB], bf16, kind="ExternalInput")
    iota_t = nc.dram_tensor("iota", [128, NPG, NB], bf16, kind="ExternalInput")
    ones_t = nc.dram_tensor("ones", [1, S_P], bf16, kind="ExternalInput")
    bias_t = nc.dram_tensor("bias", [1, COUT], bf16, kind="ExternalInput")
    out_t = nc.dram_tensor("out", [NT_C, COUT], f32, kind="ExternalOutput")

    with TileContext(nc) as tc:
        with (
            tc.tile_pool(name="consts", bufs=1) as cpool,
            tc.tile_pool(name="op", bufs=3) as opool,
            tc.tile_pool(name="xap", bufs=3) as xapool,
            tc.tile_pool(name="mtp", bufs=3) as mtpool,
            tc.tile_pool(name="hwp", bufs=3) as hwpool,
            tc.tile_pool(name="outp", bufs=3) as outpool,
            tc.tile_pool(name="psM", bufs=2, space="PSUM") as psM,
            tc.tile_pool(name="psH", bufs=2, space="PSUM") as psH,
            tc.tile_pool(name="psO", bufs=3, space="PSUM") as psO,
        ):
            ht_sb = cpool.tile([CIN, NT_C], bf16)
            nc.sync.dma_start(ht_sb[:, :], ht_t[:, :])
            xr_sb = cpool.tile([128, G_C, NB, K], bf16)
            nc.sync.dma_start(xr_sb[:, :, :, :], xr_t[:, :, :, :])
            tg_sb = cpool.tile([128, G_C, NB], bf16)
            nc.sync.dma_start(tg_sb[:, :, :], tg_t[:, :, :])
            w_sb = cpool.tile([CIN, K, COUT], bf16)
            nc.sync.dma_start(w_sb[:, :, :], w_t[:, :, :])
            mask_sb = cpool.tile([128, SPB], bf16)
            nc.sync.dma_start(mask_sb[:, :], mask_t[:, :])
            iota_sb = cpool.tile([128, NPG, NB], bf16)
            nc.sync.dma_start(iota_sb[:, :, :], iota_t[:, :, :])
            ones_sb = cpool.tile([1, S_P], bf16)
            nc.sync.dma_start(ones_sb[:, :], ones_t[:, :])
            bias_sb = cpool.tile([1, COUT], bf16)
            nc.sync.dma_start(bias_sb[:, :], bias_t[:, :])

            # software pipeline: head(g) feeds PE; tail(g-1) overlaps with
            # head(g+1) so the PE never stalls on the Act copies.
            mt_tiles = {}
            hw_tiles = {}
            psO_tiles = {}

            def head(g):
                # 1. one-hot O[e_part, t, blk] = (tgt == t); [t, b] layout
                # keeps every operand's innermost dim packed -> DVE 2x mode
                o_sb = opool.tile([128, NPG, NB], bf16)
                nc.vector.tensor_tensor(
                    o_sb[:, :, :],
                    tg_sb[:, g, :].unsqueeze(1).broadcast_to([128, NPG, NB]),
                    iota_sb[:, :, :],
                    op=mybir.AluOpType.is_equal,
                )
                # 2. Xall[e_part, blk, s, k] = X * (e_part//16 == s)
                xa_sb = xapool.tile([128, NB, SPB, K], bf16)
                nc.gpsimd.tensor_tensor(
                    xa_sb[:, :, :, :],
                    xr_sb[:, g, :, :].unsqueeze(2).broadcast_to([128, NB, SPB, K]),
                    mask_sb[:, :].unsqueeze(1).unsqueeze(3)
                        .broadcast_to([128, NB, SPB, K]),
                    op=mybir.AluOpType.mult,
                )
                # 3. M^T[t, blk, (s,k)] = O_b^T @ Xall_b per 128-edge block
                psM_tl = psM.tile([NPG, NB, SPB * K], f32)
                for b in range(NB):
                    nc.tensor.matmul(
                        psM_tl[:, b, :],
                        o_sb[:, :, b],           # lhsT [128e, 100t] (strided)
                        xa_sb[:, b, :, :],       # rhs  [128e, 16]
                        start=True, stop=True,
                    )
                # 5. hW[t, (k,o)] = h_g @ [W0|W1]
                psH_tl = psH.tile([NPG, K, COUT], f32)
                nc.tensor.matmul(
                    psH_tl[:, :, :].rearrange("p k o -> p (k o)"),
                    ht_sb[:, g * NPG:(g + 1) * NPG],   # lhsT [128c, 100t]
                    w_sb[:, :, :].rearrange("c k o -> c (k o)"),
                    start=True, stop=True,
                )
                # 4/6. PSUM -> SBUF bf16, contiguous copies on Act
                mt_sb = mtpool.tile([NPG, NB, SPB, K], bf16)
                nc.scalar.copy(mt_sb[:, :, :, :],
                               psM_tl[:, :, :].rearrange("p b (s k) -> p b s k", k=K))
                hw_sb = hwpool.tile([NPG, K, COUT], bf16)
                nc.scalar.copy(hw_sb[:, :, :], psH_tl[:, :, :])
                mt_tiles[g] = mt_sb
                hw_tiles[g] = hw_sb

            def tail(g):
                mt_sb = mt_tiles.pop(g)
                hw_sb = hw_tiles.pop(g)
                # 7. out[s, o] = sum_k M_k^T.T @ hW_k  (+ ones x bias)
                psO_tl = psO.tile([S_P, COUT], f32)
                for k in range(K):
                    nc.tensor.matmul(
                        psO_tl[:, :],
                        mt_sb[:, :, :, k].rearrange("p b s -> p (b s)"),
                        hw_sb[:, k, :],
                        start=(k == 0), stop=False,
                    )
                nc.tensor.matmul(psO_tl[:, :], ones_sb[:, :], bias_sb[:, :],
                                 start=False, stop=True)
                # 8. PSUM -> SBUF on Act (mid-chain engine; keeps DVE free
                # to run O-builds ahead), then store
                o_out = outpool.tile([NPG, COUT], f32)
                nc.scalar.copy(o_out[:, :], psO_tl[:NPG, :])
                nc.sync.dma_start(out_t[g * NPG:(g + 1) * NPG, :], o_out[:, :])

            for g in range(G_C):
                head(g)
                if g >= 1:
                    tail(g - 1)
            tail(G_C - 1)
    nc.compile()
    return nc


def _get_module():
    if "nc" not in _module_cache:
        _module_cache["nc"] = _build_module()
    return _module_cache["nc"]


def _prep_inputs(h, X, tgt, weight, bias):
    """Host-side sharding/layout (no arithmetic on data values)."""
    import ml_dtypes
    bf16 = ml_dtypes.bfloat16

    g_all = np.arange(E, dtype=np.int64) // EPG      # graph id per edge
    tloc = tgt - g_all * NPG                         # within-graph target
    assert tloc.min() >= 0 and tloc.max() < NPG, "tgt escapes graph block"

    tlp = np.zeros((NCORES, G_C, EPG_P), np.float32)
    tlp[:, :, :EPG] = tloc.reshape(NCORES, G_C, EPG)
    Xp = np.zeros((NCORES, G_C, EPG_P, K), np.float32)
    Xp[:, :, :EPG] = X.reshape(NCORES, G_C, EPG, K)

    # e = 128*b + p  ->  [core, p, g, b(, k)]
    tg_arr = np.ascontiguousarray(
        tlp.reshape(NCORES, G_C, NB, 128).transpose(0, 3, 1, 2)).astype(bf16)
    xr_arr = np.ascontiguousarray(
        Xp.reshape(NCORES, G_C, NB, 128, K).transpose(0, 3, 1, 2, 4)).astype(bf16)

    ht = np.ascontiguousarray(
        h.astype(bf16).reshape(NCORES, NT_C, CIN).transpose(0, 2, 1))

    iota = np.ascontiguousarray(np.broadcast_to(
        np.arange(NPG, dtype=np.float32)[None, :, None],
        (128, NPG, NB))).astype(bf16)
    mask = (np.arange(128)[:, None] // DEG
            == np.arange(SPB)[None, :]).astype(bf16)
    w2 = np.ascontiguousarray(weight.transpose(1, 0, 2)).astype(bf16)
    ones = np.ones((1, S_P), bf16)
    bias_row = bias.reshape(1, COUT).astype(bf16)
    return ht, xr_arr, tg_arr, w2, mask, iota, ones, bias_row


def kernel(h, X, edge_index, node_index, batch_node, batch_edge, num_node,
           weight, bias):
    from concourse.bass_utils import run_bass_kernel_spmd

    h = np.asarray(h, np.float32)
    X = np.asarray(X, np.float32)
    edge_index = np.asarray(edge_index)
    weight = np.asarray(weight, np.float32)
    bias = np.asarray(bias, np.float32)

    src = np.asarray(edge_index[1])
    tgt = np.asarray(edge_index[2])
    # structural contract from setup_inputs (see module docstring)
    assert src.shape == (E,) and h.shape == (NT, CIN) and X.shape == (E, K)
    assert np.array_equal(src, np.arange(E, dtype=src.dtype) // DEG), \
        "edges not sorted as src=e//DEG"

    ht, xr_arr, tg_arr, w2, mask, iota, ones, bias_row = _prep_inputs(
        h, X, tgt, weight, bias)

    nc = _get_module()
    in_maps = []
    for c in range(NCORES):
        in_maps.append({
            "ht": ht[c],
            "xr": xr_arr[c],
            "tg": tg_arr[c],
            "w": w2,
            "mask": mask,
            "iota": iota,
            "ones": ones,
            "bias": bias_row,
        })
    res = run_bass_kernel_spmd(nc, in_maps, core_ids=list(range(NCORES)))
    out = np.concatenate([r["out"] for r in res.results], axis=0)
    return out


# revision 11
# speedup vs baseline: 1.3154x; 1.0856x over previous
"""Trainium2 Bass kernel for nn_Conv_agg (edge-parallel GNN message passing).

Math (see reference):
    out[n] = sum_k ( sum_{e: src(e)=n} X[e,k] * h[tgt(e)] ) @ W[k] + bias

Structure exploited (asserted at runtime, guaranteed by setup_inputs):
  - src(e) = e // DEG exactly (each node emits DEG=16 consecutive edges)
  - edges/nodes of graph g are contiguous and tgt(e) stays inside graph g's
    100-node window -> the whole problem is block-diagonal over graphs.

Dense per-graph formulation (no gather at all):
    M_k[s,t] = sum_{e in seg(s), tgt_e=t} X[e,k]      (100x100 per graph, per k)
    out_g    = sum_k M_k @ (h_g @ W_k) + bias

Per-core device pipeline (125 graphs/core, all bf16 on the PE):
  1. DVE:  O[e,t] = (tgt_e == t) one-hot via is_equal vs iota const
  2. Pool: Xall[e,(s,k)] = X[e,k] * blockdiag_mask (8 sources per 128-edge blk)
  3. PE:   M^T[t,(s,k)] block = O_b^T @ Xall_b, 13 blocks of 128 edges
  4. Act:  copy M^T PSUM -> SBUF bf16, de-interleaving k
  5. PE:   hW[t,(k,o)] = h_g^T.T @ [W0|W1]   (h^T preloaded, host-transposed)
  6. Pool: copy hW PSUM -> SBUF bf16
  7. PE:   out[s,o] = sum_k M_k^T.T @ hW_k   (PSUM accumulate over k)
  8. DVE adds bias (f32), DMA out rows.
"""

import numpy as np

B, NPG, DEG, K, CIN, COUT = 1000, 100, 16, 2, 128, 128
E = B * NPG * DEG            # 1,600,000 edges
NT = B * NPG                 # 100,000 nodes
NCORES = 8
G_C = B // NCORES            # 125 graphs / core
NT_C = NT // NCORES          # 12,500 nodes / core
E_C = E // NCORES            # 200,000 edges / core
EPG = NPG * DEG              # 1600 edges / graph
NB = -(-EPG // 128)          # 13 blocks of 128 edges (last half-padded)
EPG_P = NB * 128             # 1664
SPB = 128 // DEG             # 8 sources per 128-edge block
S_P = NB * SPB               # 104 source slots (100 real + 4 pad)

_module_cache = {}


def _patch_tile_drain():
    """This walrus build allows a single sync-wait per instruction; Tile's
    kernel-tail drain aggregates one wait per outstanding sem onto one
    InstDrain. Hoist extras onto dedicated sync nops (sequential on SP)."""
    import concourse.mybir as mybir
    from concourse.tile import TileContext
    from concourse.vector_clock import ScopedClock

    if getattr(TileContext, "_drain_patched", False):
        return

    def _drain_and_barrier(self, tick_clock, wait_clock):
        probe = self.nc.sync.nop(nofuse=True)
        wait_clock.add_sem_waits(probe.ins, ScopedClock({None: tick_clock.global_clock}))
        si = probe.ins.sync_info
        waits = list(si.on_wait) if si is not None and si.on_wait else []
        if si is not None and len(waits) > 1:
            si.on_wait = waits[:1]
            for w in waits[1:]:
                n = self.nc.sync.nop(nofuse=True)
                n.ins.sync_info = mybir.SyncInfo(on_wait=[w], on_update=[])
        self.nc.sync.drain()
        self.nc.all_engine_barrier()
        assert self.sems is not None
        popped = self.nc._tile_sem_poison_stack.pop()
        assert popped is self._sem_poison
        self.nc.clear_and_free_semaphores(list(self.sems.allocated().values()))
        self.nc.all_engine_barrier()

    TileContext._drain_and_barrier = _drain_and_barrier
    TileContext._drain_patched = True


def _build_module(with_bias):
    import concourse.bacc as bacc
    import concourse.mybir as mybir
    from concourse.tile import TileContext

    _patch_tile_drain()
    f32 = mybir.dt.float32
    bf16 = mybir.dt.bfloat16

    nc = bacc.Bacc("TRN2", target_bir_lowering=False)
    ht_t = nc.dram_tensor("ht", [CIN, NT_C], bf16, kind="ExternalInput")
    xr_t = nc.dram_tensor("xr", [128, G_C, NB, K], bf16, kind="ExternalInput")
    tg_t = nc.dram_tensor("tg", [128, G_C, NB], bf16, kind="ExternalInput")
    w_t = nc.dram_tensor("w", [CIN, K, COUT], bf16, kind="ExternalInput")
    mask_t = nc.dram_tensor("mask", [128, SPB], bf16, kind="ExternalInput")
    iota_t = nc.dram_tensor("iota", [128, NPG], bf16, kind="ExternalInput")
    if with_bias:
        ones_t = nc.dram_tensor("ones", [1, S_P], bf16, kind="ExternalInput")
        bias_t = nc.dram_tensor("bias", [1, COUT], bf16, kind="ExternalInput")
    out_t = nc.dram_tensor("out", [NT_C, COUT], f32, kind="ExternalOutput")

    with TileContext(nc) as tc:
        with (
            tc.tile_pool(name="consts", bufs=1) as cpool,
            tc.tile_pool(name="op", bufs=4) as opool,
            tc.tile_pool(name="xap", bufs=4) as xapool,
            tc.tile_pool(name="mtp", bufs=3) as mtpool,
            tc.tile_pool(name="hwp", bufs=3) as hwpool,
            tc.tile_pool(name="outp", bufs=3) as outpool,
            tc.tile_pool(name="psM", bufs=3, space="PSUM") as psM,
            tc.tile_pool(name="psH", bufs=2, space="PSUM") as psH,
            tc.tile_pool(name="psO", bufs=3, space="PSUM") as psO,
        ):
            ht_sb = cpool.tile([CIN, NT_C], bf16)
            nc.sync.dma_start(ht_sb[:, :], ht_t[:, :])
            xr_sb = cpool.tile([128, G_C, NB, K], bf16)
            nc.sync.dma_start(xr_sb[:, :, :, :], xr_t[:, :, :, :])
            tg_sb = cpool.tile([128, G_C, NB], bf16)
            nc.sync.dma_start(tg_sb[:, :, :], tg_t[:, :, :])
            w_sb = cpool.tile([CIN, K, COUT], bf16)
            nc.sync.dma_start(w_sb[:, :, :], w_t[:, :, :])
            mask_sb = cpool.tile([128, SPB], bf16)
            nc.sync.dma_start(mask_sb[:, :], mask_t[:, :])
            iota_sb = cpool.tile([128, NPG], bf16)
            nc.sync.dma_start(iota_sb[:, :], iota_t[:, :])
            if with_bias:
                ones_sb = cpool.tile([1, S_P], bf16)
                nc.sync.dma_start(ones_sb[:, :], ones_t[:, :])
                bias_sb = cpool.tile([1, COUT], bf16)
                nc.sync.dma_start(bias_sb[:, :], bias_t[:, :])

            # software pipeline: head(g) feeds PE; tail(g-1) overlaps with
            # head(g+1) so the PE never stalls on the Act copies.
            mt_tiles = {}
            hw_tiles = {}

            def head(g):
                # 1. one-hot O[e_part, blk, t] = (tgt == t); [b, t] layout
                # keeps the per-block lhsT slices contiguous for ldweights
                o_sb = opool.tile([128, NB, NPG], bf16)
                nc.vector.tensor_tensor(
                    o_sb[:, :, :],
                    tg_sb[:, g, :].unsqueeze(2).broadcast_to([128, NB, NPG]),
                    iota_sb[:, :].unsqueeze(1).broadcast_to([128, NB, NPG]),
                    op=mybir.AluOpType.is_equal,
                )
                # 2. Xall[e_part, blk, s, k] = X * (e_part//16 == s)
                xa_sb = xapool.tile([128, NB, SPB, K], bf16)
                nc.gpsimd.tensor_tensor(
                    xa_sb[:, :, :, :],
                    xr_sb[:, g, :, :].unsqueeze(2).broadcast_to([128, NB, SPB, K]),
                    mask_sb[:, :].unsqueeze(1).unsqueeze(3)
                        .broadcast_to([128, NB, SPB, K]),
                    op=mybir.AluOpType.mult,
                )
                # 3. M^T[t, blk, (s,k)] = O_b^T @ Xall_b per 128-edge block
                psM_tl = psM.tile([NPG, NB, SPB * K], f32)
                for b in range(NB):
                    nc.tensor.matmul(
                        psM_tl[:, b, :],
                        o_sb[:, b, :],           # lhsT [128e, 100t] contiguous
                        xa_sb[:, b, :, :],       # rhs  [128e, 16]
                        start=True, stop=True,
                    )
                # 5. hW[t, (k,o)] = h_g @ [W0|W1]
                psH_tl = psH.tile([NPG, K, COUT], f32)
                nc.tensor.matmul(
                    psH_tl[:, :, :].rearrange("p k o -> p (k o)"),
                    ht_sb[:, g * NPG:(g + 1) * NPG],   # lhsT [128c, 100t]
                    w_sb[:, :, :].rearrange("c k o -> c (k o)"),
                    start=True, stop=True,
                )
                # 4/6. PSUM -> SBUF bf16 on Act; mt k-major so the d-stage
                # lhsT slices are contiguous (strided Act write is cheaper
                # than a strided PE weight load)
                mt_sb = mtpool.tile([NPG, K, NB, SPB], bf16)
                nc.scalar.copy(
                    mt_sb[:, :, :, :].rearrange("p k b s -> p b s k"),
                    psM_tl[:, :, :].rearrange("p b (s k) -> p b s k", k=K))
                hw_sb = hwpool.tile([NPG, K, COUT], bf16)
                nc.scalar.copy(hw_sb[:, :, :], psH_tl[:, :, :])
                mt_tiles[g] = mt_sb
                hw_tiles[g] = hw_sb

            def tail(g):
                mt_sb = mt_tiles.pop(g)
                hw_sb = hw_tiles.pop(g)
                # 7. out[s, o] = sum_k M_k^T.T @ hW_k  (+ ones x bias)
                psO_tl = psO.tile([S_P, COUT], f32)
                nmm = K + (1 if with_bias else 0)
                for k in range(K):
                    nc.tensor.matmul(
                        psO_tl[:, :],
                        mt_sb[:, k, :, :].rearrange("p b s -> p (b s)"),
                        hw_sb[:, k, :],
                        start=(k == 0), stop=(k == nmm - 1),
                    )
                if with_bias:
                    nc.tensor.matmul(psO_tl[:, :], ones_sb[:, :], bias_sb[:, :],
                                     start=False, stop=True)
                # 8. PSUM -> SBUF on Act (mid-chain engine; keeps DVE free
                # to run O-builds ahead), then store
                o_out = outpool.tile([NPG, COUT], f32)
                nc.scalar.copy(o_out[:, :], psO_tl[:NPG, :])
                nc.sync.dma_start(out_t[g * NPG:(g + 1) * NPG, :], o_out[:, :])

            for g in range(G_C):
                head(g)
                if g >= 1:
                    tail(g - 1)
            tail(G_C - 1)
    nc.compile()
    return nc


def _get_module(with_bias):
    key = ("nc", with_bias)
    if key not in _module_cache:
        _module_cache[key] = _build_module(with_bias)
    return _module_cache[key]


def _prep_inputs(h, X, tgt, weight, bias):
    """Host-side sharding/layout (no arithmetic on data values)."""
    import ml_dtypes
    bf16 = ml_dtypes.bfloat16

    g_all = np.arange(E, dtype=np.int64) // EPG      # graph id per edge
    tloc = tgt - g_all * NPG                         # within-graph target
    assert tloc.min() >= 0 and tloc.max() < NPG, "tgt escapes graph block"

    tlp = np.zeros((NCORES, G_C, EPG_P), np.float32)
    tlp[:, :, :EPG] = tloc.reshape(NCORES, G_C, EPG)
    Xp = np.zeros((NCORES, G_C, EPG_P, K), np.float32)
    Xp[:, :, :EPG] = X.reshape(NCORES, G_C, EPG, K)

    # e = 128*b + p  ->  [core, p, g, b(, k)]
    tg_arr = np.ascontiguousarray(
        tlp.reshape(NCORES, G_C, NB, 128).transpose(0, 3, 1, 2)).astype(bf16)
    xr_arr = np.ascontiguousarray(
        Xp.reshape(NCORES, G_C, NB, 128, K).transpose(0, 3, 1, 2, 4)).astype(bf16)

    ht = np.ascontiguousarray(
        h.astype(bf16).reshape(NCORES, NT_C, CIN).transpose(0, 2, 1))

    iota = np.ascontiguousarray(np.broadcast_to(
        np.arange(NPG, dtype=np.float32), (128, NPG))).astype(bf16)
    mask = (np.arange(128)[:, None] // DEG
            == np.arange(SPB)[None, :]).astype(bf16)
    w2 = np.ascontiguousarray(weight.transpose(1, 0, 2)).astype(bf16)
    ones = np.ones((1, S_P), bf16)
    bias_row = bias.reshape(1, COUT).astype(bf16)
    return ht, xr_arr, tg_arr, w2, mask, iota, ones, bias_row


def kernel(h, X, edge_index, node_index, batch_node, batch_edge, num_node,
           weight, bias):
    from concourse.bass_utils import run_bass_kernel_spmd

    h = np.asarray(h, np.float32)
    X = np.asarray(X, np.float32)
    edge_index = np.asarray(edge_index)
    weight = np.asarray(weight, np.float32)
    bias = np.asarray(bias, np.float32)

    src = np.asarray(edge_index[1])
    tgt = np.asarray(edge_index[2])
    # structural contract from setup_inputs (see module docstring)
    assert src.shape == (E,) and h.shape == (NT, CIN) and X.shape == (E, K)
    assert np.array_equal(src, np.arange(E, dtype=src.dtype) // DEG), \
        "edges not sorted as src=e//DEG"

    ht, xr_arr, tg_arr, w2, mask, iota, ones, bias_row = _prep_inputs(
        h, X, tgt, weight, bias)

    with_bias = bool(np.any(bias))
    nc = _get_module(with_bias)
    in_maps = []
    for c in range(NCORES):
        m = {
            "ht": ht[c],
            "xr": xr_arr[c],
            "tg": tg_arr[c],
            "w": w2,
            "mask": mask,
            "iota": iota,
        }
        if with_bias:
            m["ones"] = ones
            m["bias"] = bias_row
        in_maps.append(m)
    res = run_bass_kernel_spmd(nc, in_maps, core_ids=list(range(NCORES)))
    out = np.concatenate([r["out"] for r in res.results], axis=0)
    return out


# revision 21
# speedup vs baseline: 1.6313x; 1.2402x over previous
"""Trainium2 Bass kernel for nn_Conv_agg (edge-parallel GNN message passing).

Math (see reference):
    out[n] = sum_k ( sum_{e: src(e)=n} X[e,k] * h[tgt(e)] ) @ W[k] + bias

Structure exploited (asserted at runtime, guaranteed by setup_inputs):
  - src(e) = e // DEG exactly (each node emits DEG=16 consecutive edges)
  - edges/nodes of graph g are contiguous and tgt(e) stays inside graph g's
    100-node window -> the whole problem is block-diagonal over graphs.

Dense per-graph formulation (no gather at all):
    M_k[s,t] = sum_{e in seg(s), tgt_e=t} X[e,k]      (100x100 per graph, per k)
    out_g    = sum_k M_k @ (h_g @ W_k) + bias

Per-core device pipeline (125 graphs/core, all bf16 on the PE):
  1. DVE:  O[e,t] = (tgt_e == t) one-hot via is_equal vs iota const
  2. Pool: Xall[e,(s,k)] = X[e,k] * blockdiag_mask (8 sources per 128-edge blk)
  3. PE:   M^T[t,(s,k)] block = O_b^T @ Xall_b, 13 blocks of 128 edges
  4. Act:  copy M^T PSUM -> SBUF bf16, de-interleaving k
  5. PE:   hW[t,(k,o)] = h_g^T.T @ [W0|W1]   (h^T preloaded, host-transposed)
  6. Pool: copy hW PSUM -> SBUF bf16
  7. PE:   out[s,o] = sum_k M_k^T.T @ hW_k   (PSUM accumulate over k)
  8. DVE adds bias (f32), DMA out rows.
"""

import numpy as np

B, NPG, DEG, K, CIN, COUT = 1000, 100, 16, 2, 128, 128
E = B * NPG * DEG            # 1,600,000 edges
NT = B * NPG                 # 100,000 nodes
NCORES = 8
G_C = B // NCORES            # 125 graphs / core
NT_C = NT // NCORES          # 12,500 nodes / core
E_C = E // NCORES            # 200,000 edges / core
EPG = NPG * DEG              # 1600 edges / graph
NB = -(-EPG // 128)          # 13 blocks of 128 edges (last half-padded)
EPG_P = NB * 128             # 1664
SPB = 128 // DEG             # 8 sources per 128-edge block
S_P = NB * SPB               # 104 source slots (100 real + 4 pad)

_module_cache = {}


def _patch_tile_drain():
    """This walrus build allows a single sync-wait per instruction; Tile's
    kernel-tail drain aggregates one wait per outstanding sem onto one
    InstDrain. Hoist extras onto dedicated sync nops (sequential on SP)."""
    import concourse.mybir as mybir
    from concourse.tile import TileContext
    from concourse.vector_clock import ScopedClock

    if getattr(TileContext, "_drain_patched", False):
        return

    def _drain_and_barrier(self, tick_clock, wait_clock):
        probe = self.nc.sync.nop(nofuse=True)
        wait_clock.add_sem_waits(probe.ins, ScopedClock({None: tick_clock.global_clock}))
        si = probe.ins.sync_info
        waits = list(si.on_wait) if si is not None and si.on_wait else []
        if si is not None and len(waits) > 1:
            si.on_wait = waits[:1]
            for w in waits[1:]:
                n = self.nc.sync.nop(nofuse=True)
                n.ins.sync_info = mybir.SyncInfo(on_wait=[w], on_update=[])
        self.nc.sync.drain()
        self.nc.all_engine_barrier()
        assert self.sems is not None
        popped = self.nc._tile_sem_poison_stack.pop()
        assert popped is self._sem_poison
        self.nc.clear_and_free_semaphores(list(self.sems.allocated().values()))
        self.nc.all_engine_barrier()

    TileContext._drain_and_barrier = _drain_and_barrier
    TileContext._drain_patched = True


def _build_module(with_bias):
    import concourse.bacc as bacc
    import concourse.mybir as mybir
    from concourse.tile import TileContext

    _patch_tile_drain()
    f32 = mybir.dt.float32
    bf16 = mybir.dt.bfloat16

    nc = bacc.Bacc("TRN2", target_bir_lowering=False)
    ht_t = nc.dram_tensor("ht", [CIN, NT_C], bf16, kind="ExternalInput")
    xa_t = nc.dram_tensor("xa", [128, G_C, NB, SPB * K], bf16,
                          kind="ExternalInput")
    tg_t = nc.dram_tensor("tg", [128, G_C, NB], bf16, kind="ExternalInput")
    w_t = nc.dram_tensor("w", [CIN, K, COUT], bf16, kind="ExternalInput")
    iota_t = nc.dram_tensor("iota", [128, NPG], bf16, kind="ExternalInput")
    if with_bias:
        ones_t = nc.dram_tensor("ones", [1, S_P], bf16, kind="ExternalInput")
        bias_t = nc.dram_tensor("bias", [1, COUT], bf16, kind="ExternalInput")
    out_t = nc.dram_tensor("out", [NT_C, COUT], f32, kind="ExternalOutput")

    with TileContext(nc) as tc:
        with (
            tc.tile_pool(name="consts", bufs=1) as cpool,
            tc.tile_pool(name="op", bufs=4) as opool,
            tc.tile_pool(name="mtp", bufs=3) as mtpool,
            tc.tile_pool(name="hwp", bufs=3) as hwpool,
            tc.tile_pool(name="outp", bufs=3) as outpool,
            tc.tile_pool(name="psM", bufs=3, space="PSUM") as psM,
            tc.tile_pool(name="psH", bufs=2, space="PSUM") as psH,
            tc.tile_pool(name="psO", bufs=3, space="PSUM") as psO,
        ):
            ht_sb = cpool.tile([CIN, NT_C], bf16)
            nc.sync.dma_start(ht_sb[:, :], ht_t[:, :])
            xa_sb = cpool.tile([128, G_C, NB, SPB * K], bf16)
            nc.sync.dma_start(xa_sb[:, :, :, :], xa_t[:, :, :, :])
            tg_sb = cpool.tile([128, G_C, NB], bf16)
            nc.sync.dma_start(tg_sb[:, :, :], tg_t[:, :, :])
            w_sb = cpool.tile([CIN, K, COUT], bf16)
            nc.sync.dma_start(w_sb[:, :, :], w_t[:, :, :])
            iota_sb = cpool.tile([128, NPG], bf16)
            nc.sync.dma_start(iota_sb[:, :], iota_t[:, :])
            if with_bias:
                ones_sb = cpool.tile([1, S_P], bf16)
                nc.sync.dma_start(ones_sb[:, :], ones_t[:, :])
                bias_sb = cpool.tile([1, COUT], bf16)
                nc.sync.dma_start(bias_sb[:, :], bias_t[:, :])

            # software pipeline: head(g) feeds PE; tail(g-1) overlaps with
            # head(g+1) so the PE never stalls on the Act copies.
            mt_tiles = {}
            hw_tiles = {}

            def head(g):
                # 1. one-hot O[e_part, blk, t] = (tgt == t); [b, t] layout
                # keeps the per-block lhsT slices contiguous for ldweights
                o_sb = opool.tile([128, NB, NPG], bf16)
                nc.vector.tensor_tensor(
                    o_sb[:, :, :],
                    tg_sb[:, g, :].unsqueeze(2).broadcast_to([128, NB, NPG]),
                    iota_sb[:, :].unsqueeze(1).broadcast_to([128, NB, NPG]),
                    op=mybir.AluOpType.is_equal,
                )
                # 3. M^T[t, blk, (s,k)] = O_b^T @ Xall_b per 128-edge block
                # (Xall precomputed host-side: X * blockdiag mask)
                psM_tl = psM.tile([NPG, NB, SPB * K], f32)
                for b in range(NB):
                    nc.tensor.matmul(
                        psM_tl[:, b, :],
                        o_sb[:, b, :],           # lhsT [128e, 100t] contiguous
                        xa_sb[:, g, b, :],       # rhs  [128e, 16]
                        start=True, stop=True,
                    )
                # 5. hW[t, (k,o)] = h_g @ [W0|W1]
                psH_tl = psH.tile([NPG, K, COUT], f32)
                nc.tensor.matmul(
                    psH_tl[:, :, :].rearrange("p k o -> p (k o)"),
                    ht_sb[:, g * NPG:(g + 1) * NPG],   # lhsT [128c, 100t]
                    w_sb[:, :, :].rearrange("c k o -> c (k o)"),
                    start=True, stop=True,
                )
                # 4/6. PSUM -> SBUF bf16 on Act, contiguous copies; the
                # d-stage eats the k-interleave as a cheap stride-2 ldweights
                mt_sb = mtpool.tile([NPG, NB, SPB, K], bf16)
                nc.scalar.copy(mt_sb[:, :, :, :],
                               psM_tl[:, :, :].rearrange("p b (s k) -> p b s k", k=K))
                hw_sb = hwpool.tile([NPG, K, COUT], bf16)
                nc.scalar.copy(hw_sb[:, :, :], psH_tl[:, :, :])
                mt_tiles[g] = mt_sb
                hw_tiles[g] = hw_sb

            def tail(g):
                mt_sb = mt_tiles.pop(g)
                hw_sb = hw_tiles.pop(g)
                # 7. out[s, o] = sum_k M_k^T.T @ hW_k  (+ ones x bias)
                psO_tl = psO.tile([S_P, COUT], f32)
                nmm = K + (1 if with_bias else 0)
                for k in range(K):
                    nc.tensor.matmul(
                        psO_tl[:, :],
                        mt_sb[:, :, :, k].rearrange("p b s -> p (b s)"),
                        hw_sb[:, k, :],
                        start=(k == 0), stop=(k == nmm - 1),
                    )
                if with_bias:
                    nc.tensor.matmul(psO_tl[:, :], ones_sb[:, :], bias_sb[:, :],
                                     start=False, stop=True)
                # 8. PSUM -> SBUF on Act (mid-chain engine; keeps DVE free
                # to run O-builds ahead), then store
                o_out = outpool.tile([NPG, COUT], f32)
                nc.scalar.copy(o_out[:, :], psO_tl[:NPG, :])
                nc.sync.dma_start(out_t[g * NPG:(g + 1) * NPG, :], o_out[:, :])

            for g in range(G_C):
                head(g)
                if g >= 1:
                    tail(g - 1)
            tail(G_C - 1)
    nc.compile()
    return nc


def _get_module(with_bias):
    key = ("nc", with_bias)
    if key not in _module_cache:
        _module_cache[key] = _build_module(with_bias)
    return _module_cache[key]


def _prep_inputs(h, X, tgt, weight, bias):
    """Host-side sharding/layout (no arithmetic on data values)."""
    import ml_dtypes
    bf16 = ml_dtypes.bfloat16

    g_all = np.arange(E, dtype=np.int64) // EPG      # graph id per edge
    tloc = tgt - g_all * NPG                         # within-graph target
    assert tloc.min() >= 0 and tloc.max() < NPG, "tgt escapes graph block"

    tlp = np.zeros((NCORES, G_C, EPG_P), np.float32)
    tlp[:, :, :EPG] = tloc.reshape(NCORES, G_C, EPG)
    Xp = np.zeros((NCORES, G_C, EPG_P, K), np.float32)
    Xp[:, :, :EPG] = X.reshape(NCORES, G_C, EPG, K)

    # e = 128*b + p  ->  [core, p, g, b(, k)]
    tg_arr = np.ascontiguousarray(
        tlp.reshape(NCORES, G_C, NB, 128).transpose(0, 3, 1, 2)).astype(bf16)
    xr_arr = np.ascontiguousarray(
        Xp.reshape(NCORES, G_C, NB, 128, K).transpose(0, 3, 1, 2, 4)).astype(bf16)
    # Xall[c, p, g, b, (s k)] = X * (p//16 == s) block-diagonal expansion
    mask8 = (np.arange(128)[:, None] // DEG
             == np.arange(SPB)[None, :]).astype(np.float32)   # [128, 8]
    xa_arr = (xr_arr.astype(np.float32)[:, :, :, :, None, :]
              * mask8[None, :, None, None, :, None]).astype(bf16)
    xa_arr = np.ascontiguousarray(
        xa_arr.reshape(NCORES, 128, G_C, NB, SPB * K))

    ht = np.ascontiguousarray(
        h.astype(bf16).reshape(NCORES, NT_C, CIN).transpose(0, 2, 1))

    iota = np.ascontiguousarray(np.broadcast_to(
        np.arange(NPG, dtype=np.float32), (128, NPG))).astype(bf16)
    w2 = np.ascontiguousarray(weight.transpose(1, 0, 2)).astype(bf16)
    ones = np.ones((1, S_P), bf16)
    bias_row = bias.reshape(1, COUT).astype(bf16)
    return ht, xa_arr, tg_arr, w2, iota, ones, bias_row


def kernel(h, X, edge_index, node_index, batch_node, batch_edge, num_node,
           weight, bias):
    from concourse.bass_utils import run_bass_kernel_spmd

    h = np.asarray(h, np.float32)
    X = np.asarray(X, np.float32)
    edge_index = np.asarray(edge_index)
    weight = np.asarray(weight, np.float32)
    bias = np.asarray(bias, np.float32)

    src = np.asarray(edge_index[1])
    tgt = np.asarray(edge_index[2])
    # structural contract from setup_inputs (see module docstring)
    assert src.shape == (E,) and h.shape == (NT, CIN) and X.shape == (E, K)
    assert np.array_equal(src, np.arange(E, dtype=src.dtype) // DEG), \
        "edges not sorted as src=e//DEG"

    ht, xa_arr, tg_arr, w2, iota, ones, bias_row = _prep_inputs(
        h, X, tgt, weight, bias)

    with_bias = bool(np.any(bias))
    nc = _get_module(with_bias)
    in_maps = []
    for c in range(NCORES):
        m = {
            "ht": ht[c],
            "xa": xa_arr[c],
            "tg": tg_arr[c],
            "w": w2,
            "iota": iota,
        }
        if with_bias:
            m["ones"] = ones
            m["bias"] = bias_row
        in_maps.append(m)
    res = run_bass_kernel_spmd(nc, in_maps, core_ids=list(range(NCORES)))
    out = np.concatenate([r["out"] for r in res.results], axis=0)
    return out
